# revision 37
# baseline (speedup 1.0000x reference)
"""Multi-head attention (B=2, S=2048, D=1024, H=16) on 8 TRN2 NeuronCores.

Sharding: core c -> (batch b = c//4, head-group g = c%4 of 4 heads).
Each core computes, for its batch and 4 heads:
    Q/K/V projections, scores softmax (scaled by 1/sqrt(S)), attention
    output, and its partial slice of the output projection.
Host sums the 4 head-group partials per batch.

All matmuls run in float32r (fp32 with 11-bit mantissa, full PE rate).
Inputs are pre-rounded to f32r on host and X is pre-transposed so no
on-device transposes are needed:
  - Q^T/K^T [j, s] come from lhsT=W chunks, rhs=X^T chunks
  - V   [s, j]   comes from lhsT=X^T chunks, rhs=W_V chunks
  - scores S^T [k, q] from lhsT=K^T, rhs=Q^T (contraction d=64)
  - exp via ACT with fused scale, written as f32r; a ones-column in V~
    makes the PV matmul (M=65) also produce softmax denominators z
  - normalize O^T columns with 1/z via gpsimd partition_broadcast + DVE
  - Y[q, m] from lhsT=O^T chunks, rhs=W_0 chunks
"""

import sys

if "/opt/trn_rl_repo" not in sys.path:
    sys.path.insert(0, "/opt/trn_rl_repo")

import numpy as np

B = 2
S = 2048
D = 1024
H = 16
DK = 64
NCORES = 8
HG = 4  # heads per core
J = HG * DK  # 256, per-core projection width
QB = 512  # query block
NQB = S // QB  # 4
NKC = S // 128  # 16 key chunks
NDC = D // 128  # 8 contraction chunks
NJC = J // 128  # 2
SCALE_INV = float(1.0 / np.sqrt(np.float32(S)))

_CACHE = {}
LAST_RESULT = None


def _round_f32r(a: np.ndarray) -> np.ndarray:
    """Round fp32 to f32r (11-bit mantissa) with round-to-nearest-even."""
    u = np.ascontiguousarray(a, dtype=np.float32).view(np.uint32)
    r = (u + 0x7FF + ((u >> 12) & 1)) & 0xFFFFF000
    return r.astype(np.uint32).view(np.float32)


def _build():
    import concourse.mybir as mybir
    import concourse.tile as tile
    from concourse import bacc

    f32 = mybir.dt.float32
    f32r = mybir.dt.float32r

    nc = bacc.Bacc("TRN2", target_bir_lowering=False, debug=False)

    xt_d = nc.declare_dram_parameter("xt", [D, S], f32r, isOutput=False)
    wq_d = nc.declare_dram_parameter("wq", [D, J], f32r, isOutput=False)
    wk_d = nc.declare_dram_parameter("wk", [D, J], f32r, isOutput=False)
    wv_d = nc.declare_dram_parameter("wv", [D, J], f32r, isOutput=False)
    w0_d = nc.declare_dram_parameter("w0", [J, D], f32r, isOutput=False)
    y_d = nc.declare_dram_parameter("y", [S, D], f32, isOutput=True)

    with tile.TileContext(nc) as tc:
        with tc.tile_pool(name="persist", bufs=1) as A:
            # persistent tiles
            qt_t = A.tile([128, NJC, S], f32r)  # Q^T  [j, q]
            kt_t = A.tile([128, NJC, S], f32r)  # K^T  [j, k]
            v_t = A.tile([128, NKC, HG, DK + 1], f32r)  # V~ per head + ones
            ot_t = A.tile([128, NJC, S], f32r)  # O^T scaled  [j, q]
            w0_t = A.tile([128, NJC, D], f32r)
            nc.sync.dma_start(
                out=w0_t, in_=w0_d.ap().rearrange("(c p) m -> p c m", p=128)
            )
            ones_t = A.tile([128, NKC * HG], f32)
            nc.vector.memset(ones_t, 1.0)
            nc.vector.tensor_copy(out=v_t[:, :, :, DK : DK + 1], in_=ones_t)

            # ---- phase 1: load X^T / W and project ----
            # dc-outer accumulation over 8 PSUM banks so the matmul waves
            # chase the X^T chunk DMAs instead of waiting for the full 8MB.
            with (
                tc.tile_pool(name="ph1", bufs=1) as Bp,
                tc.tile_pool(name="ps1", bufs=8, space="PSUM") as psA,
            ):
                xt_t = Bp.tile([128, NDC, S], f32r)
                wq_t = Bp.tile([128, NDC, J], f32r)
                wk_t = Bp.tile([128, NDC, J], f32r)
                wv_t = Bp.tile([128, NDC, J], f32r)
                # Weights first, then X^T in dc-major waves of 8 column
                # sub-DMAs. The DMA queues round-robin and run in parallel,
                # so a wave's sub-transfers complete together -- giving
                # SEQUENTIAL per-dc arrival that the dc-outer Q-wave
                # matmuls chase chunk by chunk.
                wq_src = wq_d.ap().rearrange("(c p) j -> p c j", p=128)
                wk_src = wk_d.ap().rearrange("(c p) j -> p c j", p=128)
                xt_src = xt_d.ap().rearrange("(c p) q -> p c q", p=128)
                for dc in range(NDC):
                    nc.sync.dma_start(out=wq_t[:, dc], in_=wq_src[:, dc])
                    nc.sync.dma_start(out=xt_t[:, dc], in_=xt_src[:, dc])
                for dc in range(NDC):
                    nc.sync.dma_start(out=wk_t[:, dc], in_=wk_src[:, dc])
                nc.sync.dma_start(
                    out=wv_t, in_=wv_d.ap().rearrange("(c p) j -> p c j", p=128)
                )

                for w_t, dst in ((wq_t, qt_t), (wk_t, kt_t)):
                    tiles = [
                        psA.tile([128, QB], f32, tag="p1", name=f"p1_{i}")
                        for i in range(8)
                    ]
                    for dc in range(NDC):
                        for idx in range(8):
                            jc, qb = idx // NQB, idx % NQB
                            nc.tensor.matmul(
                                tiles[idx],
                                w_t[:, dc, jc * 128 : (jc + 1) * 128],
                                xt_t[:, dc, qb * QB : (qb + 1) * QB],
                                start=(dc == 0),
                                stop=(dc == NDC - 1),
                            )
                    for idx in range(8):
                        jc, qb = idx // NQB, idx % NQB
                        o_ap = dst[:, jc, qb * QB : (qb + 1) * QB]
                        if idx % 2 == 0:
                            nc.vector.tensor_copy(out=o_ap, in_=tiles[idx])
                        else:
                            nc.scalar.activation(
                                out=o_ap,
                                in_=tiles[idx],
                                func=mybir.ActivationFunctionType.Copy,
                                scale=1.0,
                            )
                for wave in range(2):
                    tiles = [
                        psA.tile([128, QB], f32, tag="p1", name=f"p1_{i}")
                        for i in range(8)
                    ]
                    for dc in range(NDC):
                        for idx in range(8):
                            sc = wave * 8 + idx
                            nc.tensor.matmul(
                                tiles[idx][:, 0:J],
                                xt_t[:, dc, sc * 128 : (sc + 1) * 128],
                                wv_t[:, dc, :],
                                start=(dc == 0),
                                stop=(dc == NDC - 1),
                            )
                    for idx in range(8):
                        sc = wave * 8 + idx
                        o_ap = v_t[:, sc, :, 0:DK]
                        i_ap = tiles[idx][:, 0:J].rearrange(
                            "p (h d) -> p h d", h=HG
                        )
                        if idx % 2 == 0:
                            nc.vector.tensor_copy(out=o_ap, in_=i_ap)
                        else:
                            nc.scalar.activation(
                                out=o_ap,
                                in_=i_ap,
                                func=mybir.ActivationFunctionType.Copy,
                                scale=1.0,
                            )

            # ---- phase 2+3: attention ----
            # Steps are (qb, hp) head-PAIRS, 8 total. Per step, the 32
            # score chunks (16 kc x 2 heads, interleaved kcA,kcB,...) are
            # row-packed pairs (K=64 at base partitions 0/64 run
            # concurrently at full-array rate). Score PSUM tiles hold 3
            # chunks (6 banks double-buffered) so one ACT exp op covers
            # 1536 elements and the exp stream runs back-to-back -- ACT is
            # the pacing engine. exp results go to a per-step persistent
            # expst tile; the previous step's 32 PV matmuls (dense K=128
            # full-array work, which also re-warms the HAM clock gate) are
            # emitted as a clump at the start of the next step, in chunk
            # order so expst chunks free up for the incoming exp stream.
            # The output projection runs as a tail phase in its own pool.
            with (
                tc.tile_pool(name="work", bufs=1) as C,
                tc.tile_pool(name="nrm", bufs=2) as Cn,
                tc.tile_pool(name="dbounce", bufs=2, space="DRAM") as Cd,
                tc.tile_pool(name="ps_s", bufs=2, space="PSUM") as psS,
                tc.tile_pool(name="ps_o", bufs=1, space="PSUM") as psO,
            ):
                NCH = 2 * NKC  # 32 score chunks per step
                GRPS = [(0, 2), (2, 5), (5, 8), (8, 11), (11, 14),
                        (14, 17), (17, 20), (20, 23), (23, 26), (26, 29),
                        (29, 32)]

                def emit_scores(qb, hp):
                    """Packed score pairs + exp stream -> per-step expst."""
                    q_sl = slice(qb * QB, (qb + 1) * QB)
                    expst = C.tile([128, NCH, QB], f32r, tag="expst")
                    for g0, g1 in GRPS:
                        ps = psS.tile([128, 3, QB], f32, tag="s")
                        for i, c in enumerate(range(g0, g1)):
                            kc, hb = c // 2, c % 2
                            p0 = hb * 64
                            k_sl = slice(kc * 128, (kc + 1) * 128)
                            nc.tensor.matmul(
                                ps[:, i],
                                kt_t[p0 : p0 + 64, hp, k_sl],
                                qt_t[p0 : p0 + 64, hp, q_sl],
                                start=True,
                                stop=True,
                                tile_position=(p0, 0),
                            )
                        nc.scalar.activation(
                            out=expst[:, g0:g1, :],
                            in_=ps[:, 0 : g1 - g0, :],
                            func=mybir.ActivationFunctionType.Exp,
                            scale=SCALE_INV,
                        )
                    return expst

                def emit_pv(qb, hp, expst):
                    """Dense PV clump: 32 K=128 matmuls in chunk order."""
                    ps_oa = psO.tile([128, QB], f32, tag="oa")
                    ps_ob = psO.tile([128, QB], f32, tag="ob")
                    for c in range(NCH):
                        kc, hb = c // 2, c % 2
                        ps_o = ps_oa if hb == 0 else ps_ob
                        nc.tensor.matmul(
                            ps_o[0 : DK + 1, :],
                            v_t[:, kc, 2 * hp + hb, :],
                            expst[:, c, :],
                            start=(kc == 0),
                            stop=(kc == NKC - 1),
                        )
                    return ps_oa, ps_ob

                def emit_normalize(qb, hp, ps_oa, ps_ob):
                    # Copy O rows to SBUF immediately (releases the PSUM
                    # accumulator); bounce z through DRAM to reshape
                    # [1, QB] -> [128, QB/128] so the DVE's iterative
                    # reciprocal runs on 128 lanes instead of one, then
                    # broadcast 1/z back via a partition-step-0 DRAM read.
                    q_sl = slice(qb * QB, (qb + 1) * QB)
                    for p0, ps_o in ((0, ps_oa), (64, ps_ob)):
                        o_sb = Cn.tile([DK, QB], f32, tag=f"osb{p0}")
                        nc.vector.tensor_copy(o_sb, ps_o[0:DK, :])
                        z_sb = Cn.tile([1, QB], f32, tag=f"zs{p0}")
                        nc.vector.tensor_copy(z_sb, ps_o[DK : DK + 1, :])
                        z_dr = Cd.tile([1, QB], f32, tag=f"zd{p0}")
                        nc.sync.dma_start(out=z_dr, in_=z_sb)
                        z128 = Cn.tile([128, QB // 128, 1], f32, tag=f"z{p0}")
                        nc.sync.dma_start(
                            out=z128,
                            in_=z_dr.rearrange("a (p i) -> (a p) i", p=128),
                        )
                        r128 = Cn.tile([128, QB // 128, 1], f32, tag=f"rc{p0}")
                        nc.vector.reciprocal(r128, z128)
                        r_dr = Cd.tile([1, QB], f32, tag=f"rd{p0}")
                        nc.sync.dma_start(
                            out=r_dr.rearrange("a (p i) -> (a p) i", p=128),
                            in_=r128,
                        )
                        r_b = Cn.tile([64, QB], f32, tag=f"rb{p0}")
                        nc.sync.dma_start(
                            out=r_b, in_=r_dr.to_broadcast([64, QB])
                        )
                        nc.vector.tensor_mul(
                            ot_t[p0 : p0 + 64, hp, q_sl],
                            o_sb,
                            r_b,
                        )

                def emit_step(qb, hp, prev):
                    """Scores+exp stream for (qb, hp); the previous step's
                    PV chunks ride between score groups so the ACT exp
                    stream never pauses at step boundaries."""
                    q_sl = slice(qb * QB, (qb + 1) * QB)
                    expst = C.tile([128, NCH, QB], f32r, tag="expst")
                    if prev is not None:
                        pq, php, pexp = prev
                        ps_oa = psO.tile([128, QB], f32, tag="oa")
                        ps_ob = psO.tile([128, QB], f32, tag="ob")
                    def pv_half(c0, c1):
                        # dense half-clump of K=128 matmuls: feeds the PV
                        # accumulators AND keeps the HAM clock-gate warm
                        for c in range(c0, c1):
                            kc, hb = c // 2, c % 2
                            ps_o = ps_oa if hb == 0 else ps_ob
                            nc.tensor.matmul(
                                ps_o[0 : DK + 1, :],
                                v_t[:, kc, 2 * php + hb, :],
                                pexp[:, c, :],
                                start=(kc == 0),
                                stop=(kc == NKC - 1),
                            )

                    def score_group(g0, g1):
                        ps = psS.tile([128, 3, QB], f32, tag="s", name="ps_s")
                        for i, c in enumerate(range(g0, g1)):
                            kc, hb = c // 2, c % 2
                            p0 = hb * 64
                            k_sl = slice(kc * 128, (kc + 1) * 128)
                            nc.tensor.matmul(
                                ps[:, i],
                                kt_t[p0 : p0 + 64, hp, k_sl],
                                qt_t[p0 : p0 + 64, hp, q_sl],
                                start=True,
                                stop=True,
                                tile_position=(p0, 0),
                            )
                        nc.scalar.activation(
                            out=expst[:, g0:g1, :],
                            in_=ps[:, 0 : g1 - g0, :],
                            func=mybir.ActivationFunctionType.Exp,
                            scale=SCALE_INV,
                        )

                    # previous step's full dense PV clump first (re-warms
                    # the HAM clock gate), then this step's score groups
                    if prev is not None:
                        pv_half(0, NCH)
                    for g0, g1 in GRPS:
                        score_group(g0, g1)
                    if prev is not None:
                        emit_normalize(pq, php, ps_oa, ps_ob)
                    return expst

                steps = [(qb, hp) for qb in range(NQB) for hp in range(2)]
                prev = None
                for qb, hp in steps:
                    expst = emit_step(qb, hp, prev)
                    prev = (qb, hp, expst)
                pq, php, pexp = prev
                ps_oa, ps_ob = emit_pv(pq, php, pexp)
                emit_normalize(pq, php, ps_oa, ps_ob)

            # ---- phase 4: output projection tail ----
            with (
                tc.tile_pool(name="ytail", bufs=8) as Cy,
                tc.tile_pool(name="ps_yt", bufs=8, space="PSUM") as psY,
            ):
                for qc in range(S // 128):
                    for mb in range(D // QB):
                        ps = psY.tile([128, QB], f32, tag="y")
                        for jc in range(NJC):
                            nc.tensor.matmul(
                                ps,
                                ot_t[:, jc, qc * 128 : (qc + 1) * 128],
                                w0_t[:, jc, mb * QB : (mb + 1) * QB],
                                start=(jc == 0),
                                stop=(jc == NJC - 1),
                            )
                        y_t = Cy.tile([128, QB], f32, tag="yt")
                        if (qc + mb) % 2 == 0:
                            nc.vector.tensor_copy(out=y_t, in_=ps)
                        else:
                            nc.scalar.activation(
                                out=y_t,
                                in_=ps,
                                func=mybir.ActivationFunctionType.Copy,
                                scale=1.0,
                            )
                        nc.sync.dma_start(
                            out=y_d.ap()[
                                qc * 128 : (qc + 1) * 128,
                                mb * QB : (mb + 1) * QB,
                            ],
                            in_=y_t,
                        )


    nc.compile()
    return nc


def kernel(X, W_Q, W_K, W_V, W_0):
    global LAST_RESULT
    from concourse.bass_utils import run_bass_kernel_spmd
    import os

    X = np.asarray(X, dtype=np.float32)
    W_Q = np.asarray(W_Q, dtype=np.float32)
    W_K = np.asarray(W_K, dtype=np.float32)
    W_V = np.asarray(W_V, dtype=np.float32)
    W_0 = np.asarray(W_0, dtype=np.float32)

    if "nc" not in _CACHE:
        _CACHE["nc"] = _build()
    nc = _CACHE["nc"]

    xt = [_round_f32r(X[b].T) for b in range(B)]
    in_maps = []
    for c in range(NCORES):
        b, g = c // HG, c % HG
        js = slice(g * J, (g + 1) * J)
        in_maps.append(
            {
                "xt": xt[b],
                "wq": _round_f32r(W_Q[:, js]),
                "wk": _round_f32r(W_K[:, js]),
                "wv": _round_f32r(W_V[:, js]),
                "w0": _round_f32r(W_0[js, :]),
            }
        )

    trace = bool(int(os.environ.get("KERNEL_TRACE", "0")))
    res = run_bass_kernel_spmd(
        nc, in_maps, list(range(NCORES)), trace=trace
    )
    LAST_RESULT = res

    out = np.zeros((B, S, D), dtype=np.float32)
    for c in range(NCORES):
        out[c // HG] += res.results[c]["y"]
    return out
